# revision 51
# baseline (speedup 1.0000x reference)
"""Transformer block (pre-LN attention + FFN) on 8 TRN2 NeuronCores — v3.

Sharding (core c of 8): attention heads {2c, 2c+1} for BOTH batches;
own global token block c (batch c//4, tokens [512*(c%4), +512)) for
proj/LN2/FFN/residual/output.

  - LN1 replicated per batch on every core (no AllGather); batch 1's LN and
    QKV are emitted interleaved into batch 0's attention so every engine
    queue stays busy.
  - One 8-core AllToAll (fp8, split into two token-halves, CC path
    pre-warmed by a dummy collective) moves transposed attention features;
    proj/LN2/FFN run fully local (no ReduceScatter).
  - All weights host-pre-cast and PRE-ARRANGED into the on-chip layouts so
    every DMA is a cheap contiguous descriptor.
  - FFN runs in fp8 (weights host-scaled x16) with DoubleRow perf mode.
  - LN stats are grouped (4 chunks) to minimize ACT table reloads.
  - Output produced transposed ([C, 512] per core), untransposed on host.
"""

import numpy as np

import concourse.bass as bass
import concourse.mybir as mybir
import concourse.tile as tile
from concourse import bacc
from concourse.bass_utils import run_bass_kernel_spmd
from concourse.masks import make_identity

P = 128
C = 1024          # n_embd
KT = C // P       # 8 c-tiles
T = 2048          # tokens per batch
NTC = T // P      # 16 token chunks per batch
TOWN = 512        # own tokens per core
D = 64            # head dim
FF = 4096
FMT = FF // P     # 32 ffn m-tiles
CH = 256          # attention query chunk
QC = T // CH      # 8 chunks
EPS = 1e-5
SCALE = 1.0 / 32.0  # C ** -0.5
W8S = 16.0          # host-side fp8 weight scale for W1/W2
GROUP8 = [[0, 1, 2, 3, 4, 5, 6, 7]]
NCORES = 8

f32 = mybir.dt.float32
bf16 = mybir.dt.bfloat16
f8 = mybir.dt.float8e4
AX = mybir.AxisListType
ALU = mybir.AluOpType
ACT_F = mybir.ActivationFunctionType
DR = mybir.MatmulPerfMode.DoubleRow


def build():
    nc = bacc.Bacc("TRN2", target_bir_lowering=False, debug=False,
                   num_devices=NCORES)
    _build_graph(nc)
    nc.compile()
    return nc


def _build_graph(nc):
    xb_ext = nc.dram_tensor("xb", [2, T, C], bf16, kind="ExternalInput").ap()
    xo_ext = nc.dram_tensor("xo", [P, 4, C], f32, kind="ExternalInput").ap()
    wq_ext = nc.dram_tensor("wq", [P, KT, P], bf16, kind="ExternalInput").ap()
    wk_ext = nc.dram_tensor("wk", [P, KT, P], bf16, kind="ExternalInput").ap()
    wv_ext = nc.dram_tensor("wv", [P, KT, P], bf16, kind="ExternalInput").ap()
    wp_ext = nc.dram_tensor("wp", [P, KT, C], bf16, kind="ExternalInput").ap()
    w1_ext = nc.dram_tensor("w1", [FMT, P, KT, P], bf16,
                            kind="ExternalInput").ap()
    w2_ext = nc.dram_tensor("w2", [KT, P, FMT, P], bf16,
                            kind="ExternalInput").ap()
    bq_ext = nc.dram_tensor("bq", [P, 1], f32, kind="ExternalInput").ap()
    b1_ext = nc.dram_tensor("b1", [P, FMT], f32, kind="ExternalInput").ap()
    b2_ext = nc.dram_tensor("b2", [P, KT], f32, kind="ExternalInput").ap()
    outT_ext = nc.dram_tensor("outT", [C, TOWN], f32,
                              kind="ExternalOutput").ap()

    with tile.TileContext(nc) as tc:
        with (
            tc.tile_pool(name="sb", bufs=1) as sb,
            tc.tile_pool(name="st", bufs=3) as st,
            tc.tile_pool(name="ps", bufs=1, space="PSUM") as ps,
            tc.tile_pool(name="dram", bufs=1, space="DRAM") as dram,
        ):
            # ---- constants ----
            id_bf = sb.tile([P, P], bf16)
            make_identity(nc, id_bf[:])
            id_f32 = sb.tile([P, P], f32)
            make_identity(nc, id_f32[:])
            # causal mask for diagonal blocks, layout [key_p, hl, query]
            # per key-shift sh: keep where key (128*sh + p) <= query y
            mask = sb.tile([P, 2, 2, CH], bf16)
            nc.gpsimd.memset(mask[:], 1.0)
            nc.gpsimd.affine_select(
                out=mask[:], in_=mask[:], compare_op=ALU.is_ge, fill=0.0,
                base=0, pattern=[[-P, 2], [0, 2], [1, CH]],
                channel_multiplier=-1)

            # ---- CC warmup: tiny AllToAll so the real ones start fast.
            # GpSimd has nothing else queued before the real triggers, so
            # this can block its queue harmlessly while absorbing skew.
            warm_in = dram.tile([NCORES * P, 64], f8, name="warm_in")
            warm_out = dram.tile([NCORES * P, 64], f8, name="warm_out")
            warm_sb = sb.tile([P, 64], f8)
            nc.vector.memset(warm_sb[:], 0.0)
            for i in range(NCORES):
                nc.sync.dma_start(warm_in[i * P:(i + 1) * P, :], warm_sb[:])
            nc.gpsimd.collective_compute(
                "AllToAll", ALU.bypass, ins=[warm_in.opt()],
                outs=[warm_out.opt()], replica_groups=GROUP8)

            # LN1 per-token stats, one column per (batch, token chunk)
            ssum = sb.tile([P, 2 * NTC], f32)
            sqs = sb.tile([P, 2 * NTC], f32)
            mu = sb.tile([P, 2 * NTC], f32)
            rstd = sb.tile([P, 2 * NTC], f32)
            nvar = sb.tile([P, 2 * NTC], f32)

            xbc_tiles = {}

            def ln1_stats_chunk(b, tci):
                """DMA chunk, accumulate sum and sum-of-squares.
                Row-sums via STT-with-accum (16-bit in/out)."""
                s = slice(b * NTC + tci, b * NTC + tci + 1)
                xbc = st.tile([P, C], bf16, tag="xb", bufs=8,
                              name=f"xbc{b}_{tci}")
                xbc_tiles[(b, tci)] = xbc
                nc.sync.dma_start(xbc[:], xb_ext[b, tci * P:(tci + 1) * P, :])
                so = st.tile([P, C], bf16, tag="sq", bufs=2,
                             name=f"so{b}_{tci}")
                nc.vector.scalar_tensor_tensor(
                    out=so[:], in0=xbc[:], scalar=0.0, in1=xbc[:],
                    op0=ALU.add, op1=ALU.bypass, accum_out=ssum[:, s])
                if b == 0:
                    sqo = st.tile([P, C], bf16, tag="sq", bufs=2,
                                  name=f"sqo{b}_{tci}")
                    nc.scalar.activation(sqo[:], xbc[:], ACT_F.Square,
                                         accum_out=sqs[:, s])
                else:
                    sqo = st.tile([P, C], bf16, tag="sq", bufs=2,
                                  name=f"sqo{b}_{tci}")
                    nc.vector.scalar_tensor_tensor(
                        out=sqo[:], in0=xbc[:], scalar=1.0, in1=xbc[:],
                        op0=ALU.mult, op1=ALU.mult, accum_out=sqs[:, s])

            def ln_group_stats(sl):
                """Batched stats for a group of chunk columns sl."""
                nc.vector.tensor_scalar(
                    out=mu[:, sl], in0=ssum[:, sl], scalar1=1.0 / C,
                    scalar2=None, op0=ALU.mult)
                nc.vector.tensor_tensor(out=nvar[:, sl], in0=mu[:, sl],
                                        in1=mu[:, sl], op=ALU.mult)
                nc.vector.scalar_tensor_tensor(
                    out=nvar[:, sl], in0=sqs[:, sl], scalar=1.0 / C,
                    in1=nvar[:, sl], op0=ALU.mult, op1=ALU.subtract)
                nc.vector.tensor_scalar(
                    out=nvar[:, sl], in0=nvar[:, sl], scalar1=EPS,
                    scalar2=None, op0=ALU.add)
                nc.vector.reciprocal(nvar[:, sl], nvar[:, sl])
                nc.scalar.sqrt(rstd[:, sl], nvar[:, sl])

            def ln1_apply_chunk(b, tci, hT):
                """Normalize chunk and transpose into hT (packed evac)."""
                s = slice(b * NTC + tci, b * NTC + tci + 1)
                xbc = xbc_tiles.pop((b, tci))
                hc = st.tile([P, C], bf16, tag="h", bufs=2,
                             name=f"hc{b}_{tci}")
                nc.vector.tensor_scalar(
                    out=hc[:], in0=xbc[:], scalar1=mu[:, s],
                    scalar2=rstd[:, s], op0=ALU.subtract, op1=ALU.mult)
                tp = ps.tile([P, KT, P], bf16, tag="tp", bufs=1,
                             name=f"tph{b}_{tci}")
                for kt in range(KT):
                    nc.tensor.transpose(tp[:, kt, :],
                                        hc[:, kt * P:(kt + 1) * P],
                                        id_bf[:])
                nc.vector.tensor_copy(
                    out=hT[:, :, tci * P:(tci + 1) * P], in_=tp[:])

            def qkv_k(b, hT, kT, w_sb, th):
                pp = ps.tile([P, TOWN], f32, tag="big", bufs=3,
                             name=f"k{b}_{th}")
                for kt in range(KT):
                    nc.tensor.matmul(
                        pp[:], w_sb[:, kt, :],
                        hT[:, kt, th * TOWN:(th + 1) * TOWN],
                        start=(kt == 0), stop=(kt == KT - 1))
                ts_ = slice(th * TOWN, (th + 1) * TOWN)
                nc.vector.tensor_copy(out=kT[0:D, 0, ts_], in_=pp[0:D, :])
                nc.vector.tensor_copy(out=kT[D:P, 1, ts_], in_=pp[D:P, :])

            def qkv_q(b, hT, qT, w_sb, bq_sb, th):
                pp = ps.tile([P, TOWN], f32, tag="big", bufs=3,
                             name=f"q{b}_{th}")
                for kt in range(KT):
                    nc.tensor.matmul(
                        pp[:], w_sb[:, kt, :],
                        hT[:, kt, th * TOWN:(th + 1) * TOWN],
                        start=(kt == 0), stop=(kt == KT - 1))
                nc.vector.tensor_scalar(
                    out=qT[:, th * TOWN:(th + 1) * TOWN], in0=pp[:],
                    scalar1=bq_sb[:], scalar2=None, op0=ALU.add)

            def qkv_v(b, hT, v_aug, w_sb, tci):
                pp = ps.tile([P, P], f32, tag="tp", bufs=1,
                             name=f"v{b}_{tci}")
                for kt in range(KT):
                    nc.tensor.matmul(
                        pp[:], hT[:, kt, tci * P:(tci + 1) * P],
                        w_sb[:, kt, :],
                        start=(kt == 0), stop=(kt == KT - 1))
                nc.vector.tensor_copy(
                    out=v_aug[:, tci, :, 0:D],
                    in_=pp[:].rearrange("p (h d) -> p h d", d=D))

            def attn_qc(b, qc, kT, qT, v_aug, attn_sb):
                """Scores+exp for ALL key chunks first (keeps the scalar
                engine's exp stream continuous), then AV matmuls with V as
                the stationary operand (one weight load per key chunk, wide
                moving operand) producing feature-major partials [65, 256]
                that are transposed+normalized at evacuation."""
                if True:
                    apsT = [ps.tile([P, CH], f32, tag="aps", bufs=4,
                                    name=f"apsT{b}_{qc}_{hl}")
                            for hl in range(2)]
                    pend = []

                    def flush_avs():
                        for kc_, sh_, ex_ in pend:
                            for hl in range(2):
                                nc.tensor.matmul(
                                    apsT[hl][:],
                                    v_aug[:, 2 * kc_ + sh_, hl, :],
                                    ex_[:, hl, :],
                                    start=(kc_ == 0 and sh_ == 0),
                                    stop=(kc_ == qc and sh_ == 1))
                        pend.clear()

                    for kc in range(qc + 1):
                        for sh in range(2):
                            sc = ps.tile([P, 2, CH], f32, tag="big", bufs=3,
                                         name=f"sc{b}_{qc}_{kc}_{sh}")
                            for hl in range(2):
                                nc.tensor.matmul(
                                    sc[:, hl, :],
                                    kT[:, hl,
                                       kc * CH + sh * P:kc * CH + (sh + 1) * P],
                                    qT[:, qc * CH:(qc + 1) * CH],
                                    start=True, stop=True)
                            ex = st.tile([P, 2, CH], bf16, tag="ex", bufs=9,
                                         name=f"ex{b}_{qc}_{kc}_{sh}")
                            nc.scalar.activation(ex[:], sc[:], ACT_F.Exp,
                                                 bias=0.0, scale=SCALE)
                            if kc == qc:
                                nc.vector.tensor_tensor(
                                    out=ex[:], in0=ex[:], in1=mask[:, sh],
                                    op=ALU.mult)
                            pend.append((kc, sh, ex))
                        if len(pend) >= 8:
                            flush_avs()
                    flush_avs()
                    for hl in range(2):
                        avs = st.tile([P, CH], bf16, tag="avs", bufs=3,
                                      name=f"avs{b}_{qc}_{hl}")
                        nc.vector.tensor_copy(out=avs[:], in_=apsT[hl][:])
                        for ti in range(2):
                            tp = ps.tile([P, P], bf16, tag="tp", bufs=1,
                                         name=f"tpav{b}_{qc}_{hl}_{ti}")
                            nc.tensor.transpose(
                                tp[:], avs[:, ti * P:(ti + 1) * P], id_bf[:])
                            rd = st.tile([P, 1], f32, tag="rd", bufs=4,
                                         name=f"rd{b}_{qc}_{ti}_{hl}")
                            nc.vector.reciprocal(rd[:], tp[:, D:D + 1])
                            nc.vector.tensor_scalar(
                                out=attn_sb[:, 2 * qc + ti,
                                            hl * D:(hl + 1) * D],
                                in0=tp[:, 0:D], scalar1=rd[:],
                                scalar2=None, op0=ALU.mult)

            # ---- persistent tiles ----
            hT_0 = sb.tile([P, KT, T], bf16, tag="TA", name="hT_0")
            kT_0 = sb.tile([P, 2, T], bf16, tag="TK0", name="kT_0")
            qT_0 = sb.tile([P, T], bf16, name="qT_0")
            va_0 = sb.tile([P, NTC, 2, P], bf16, name="va_0")
            hT_1 = sb.tile([P, KT, T], bf16, tag="TA", name="hT_1")
            kT_1 = sb.tile([P, 2, T], bf16, name="kT_1")
            qT_1 = sb.tile([P, T], bf16, name="qT_1")
            va_1 = sb.tile([P, NTC, 2, P], bf16, name="va_1")
            at_0 = sb.tile([P, NTC, P], bf16, name="at_0")
            at_1 = sb.tile([P, NTC, P], bf16, name="at_1")
            afT = sb.tile([P, KT, TOWN], f8, name="afT")
            out1 = sb.tile([P, 4, C], f32, name="out1")
            h2T = sb.tile([P, KT, TOWN], bf16, name="h2T")

            # ---- weights (contiguous, host-prearranged) ----
            wq_sb = sb.tile([P, KT, P], bf16)
            wk_sb = sb.tile([P, KT, P], bf16)
            wv_sb = sb.tile([P, KT, P], bf16)
            bq_sb = sb.tile([P, 1], f32)
            for w_sb, ext in ((wk_sb, wk_ext), (wq_sb, wq_ext),
                              (wv_sb, wv_ext), (bq_sb, bq_ext)):
                nc.sync.dma_start(w_sb[:], ext[:])

            # ---- unified schedule: LN1/QKV/attention for both batches ----
            # Early attention query-chunks are emitted as soon as the K/Q/V
            # slices they read exist, keeping tensor/vector/scalar all busy.
            nc.vector.memset(kT_0[D:P, 0, :], 0.0)
            nc.vector.memset(kT_0[0:D, 1, :], 0.0)
            nc.vector.memset(va_0[:, :, :, D:P], 0.0)
            nc.vector.memset(va_0[:, :, :, D:D + 1], 1.0)

            def stats(b, lo, hi):
                for tci in range(lo, hi):
                    ln1_stats_chunk(b, tci)

            def finish(b, lo, hi):
                ln_group_stats(slice(b * NTC + lo, b * NTC + hi))

            def apply(b, lo, hi, hT):
                for tci in range(lo, hi):
                    ln1_apply_chunk(b, tci, hT)

            def vs(b, hT, va, w_sb, lo, hi):
                for tci in range(lo, hi):
                    qkv_v(b, hT, va, w_sb, tci)

            # a2a input buffers (fp8, asymmetric token split: the first
            # collective pays the CC ramp, so keep it small)
            A2A0 = 128
            a2a_in = [dram.tile([NCORES * P, A2A0], f8, name="a2ain0"),
                      dram.tile([NCORES * P, TOWN - A2A0], f8,
                                name="a2ain1")]
            a2a_out = [dram.tile([NCORES * P, A2A0], f8, name="a2aout0"),
                       dram.tile([NCORES * P, TOWN - A2A0], f8,
                                 name="a2aout1")]

            def a2a_send_block(b, attn_sb, ib):
                # transpose one 512-token block of batch-b attention and
                # stage it; global block i = b*4 + ib -> rows i*128..
                i = b * 4 + ib
                atT = st.tile([P, TOWN], f8, tag="atT", bufs=2, name=f"atT{i}")
                tp = ps.tile([P, 4, P], bf16, tag="tp", bufs=1,
                             name=f"tpa{i}")
                for tt in range(4):
                    nc.tensor.transpose(tp[:, tt, :],
                                        attn_sb[:, ib * 4 + tt, :],
                                        id_bf[:])
                nc.vector.tensor_copy(out=atT[:], in_=tp[:])
                nc.sync.dma_start(a2a_in[0][i * P:(i + 1) * P, :],
                                  atT[:, 0:A2A0])
                nc.sync.dma_start(a2a_in[1][i * P:(i + 1) * P, :],
                                  atT[:, A2A0:TOWN])

            def A0(qc):
                attn_qc(0, qc, kT_0, qT_0, va_0, at_0)

            def A1(qc):
                attn_qc(1, qc, kT_1, qT_1, va_1, at_1)

            # schedule: one attention qc per slot; the LN/QKV/a2a build work
            # is diced finely between slots so the exp stream never starves
            stats(0, 0, 4)
            finish(0, 0, 4)
            apply(0, 0, 4, hT_0)
            stats(0, 4, 8)
            qkv_k(0, hT_0, kT_0, wk_sb, 0)
            qkv_q(0, hT_0, qT_0, wq_sb, bq_sb, 0)
            vs(0, hT_0, va_0, wv_sb, 0, 4)
            A0(0)
            finish(0, 4, 8)
            apply(0, 4, 8, hT_0)
            A0(1)
            a2a_send_block(0, at_0, 0)
            qkv_k(0, hT_0, kT_0, wk_sb, 1)
            qkv_q(0, hT_0, qT_0, wq_sb, bq_sb, 1)
            vs(0, hT_0, va_0, wv_sb, 4, 8)
            A0(2)
            stats(0, 8, 12)
            A0(3)
            a2a_send_block(0, at_0, 1)
            stats(0, 12, 16)
            finish(0, 8, 16)
            apply(0, 8, 12, hT_0)
            qkv_k(0, hT_0, kT_0, wk_sb, 2)
            qkv_q(0, hT_0, qT_0, wq_sb, bq_sb, 2)
            vs(0, hT_0, va_0, wv_sb, 8, 12)
            A0(4)
            apply(0, 12, 16, hT_0)
            qkv_k(0, hT_0, kT_0, wk_sb, 3)
            qkv_q(0, hT_0, qT_0, wq_sb, bq_sb, 3)
            A0(5)
            a2a_send_block(0, at_0, 2)
            vs(0, hT_0, va_0, wv_sb, 12, 16)
            stats(1, 0, 4)
            A0(6)
            stats(1, 4, 8)
            finish(1, 0, 8)
            apply(1, 0, 4, hT_1)
            A0(7)
            a2a_send_block(0, at_0, 3)
            apply(1, 4, 8, hT_1)
            nc.vector.memset(kT_1[D:P, 0, :], 0.0)
            nc.vector.memset(kT_1[0:D, 1, :], 0.0)
            nc.vector.memset(va_1[:, :, :, D:P], 0.0)
            nc.vector.memset(va_1[:, :, :, D:D + 1], 1.0)
            qkv_k(1, hT_1, kT_1, wk_sb, 0)
            qkv_q(1, hT_1, qT_1, wq_sb, bq_sb, 0)
            vs(1, hT_1, va_1, wv_sb, 0, 4)
            stats(1, 8, 12)

            # remaining weights (needed much later)
            wp_sb = sb.tile([P, KT, C], bf16)
            nc.sync.dma_start(wp_sb[:], wp_ext[:])
            xo_sb = sb.tile([P, 4, C], f32)
            nc.sync.dma_start(xo_sb[:], xo_ext[:])
            b1_sb = sb.tile([P, FMT], f32)
            nc.sync.dma_start(b1_sb[:], b1_ext[:])
            b2_sb = sb.tile([P, KT], f32)
            nc.sync.dma_start(b2_sb[:], b2_ext[:])

            stats(1, 12, 16)
            finish(1, 8, 16)
            apply(1, 8, 12, hT_1)
            qkv_k(1, hT_1, kT_1, wk_sb, 1)
            qkv_q(1, hT_1, qT_1, wq_sb, bq_sb, 1)
            vs(1, hT_1, va_1, wv_sb, 4, 8)
            A1(0)
            apply(1, 12, 16, hT_1)
            qkv_k(1, hT_1, kT_1, wk_sb, 2)
            qkv_q(1, hT_1, qT_1, wq_sb, bq_sb, 2)
            A1(1)
            a2a_send_block(1, at_1, 0)
            vs(1, hT_1, va_1, wv_sb, 8, 12)
            A1(2)
            qkv_k(1, hT_1, kT_1, wk_sb, 3)
            qkv_q(1, hT_1, qT_1, wq_sb, bq_sb, 3)
            A1(3)
            a2a_send_block(1, at_1, 1)
            vs(1, hT_1, va_1, wv_sb, 12, 16)
            A1(4)
            A1(5)
            a2a_send_block(1, at_1, 2)
            A1(6)
            A1(7)
            a2a_send_block(1, at_1, 3)
            for h in range(2):
                nc.gpsimd.collective_compute(
                    "AllToAll", ALU.bypass, ins=[a2a_in[h].opt()],
                    outs=[a2a_out[h].opt()], replica_groups=GROUP8)
            for s_i in range(NCORES):
                nc.sync.dma_start(afT[:, s_i, 0:A2A0],
                                  a2a_out[0][s_i * P:(s_i + 1) * P, :])
            for s_i in range(NCORES):
                nc.sync.dma_start(afT[:, s_i, A2A0:TOWN],
                                  a2a_out[1][s_i * P:(s_i + 1) * P, :])

            # ---- proj + residual (own tokens, token-major) ----
            for m in range(4):
                for nh in range(2):
                    cs = slice(nh * TOWN, (nh + 1) * TOWN)
                    pp = ps.tile([P, TOWN], f32, tag="big", bufs=3,
                                 name=f"proj{m}_{nh}")
                    for fc in range(KT):
                        nc.tensor.matmul(
                            pp[:], afT[:, fc, m * P:(m + 1) * P],
                            wp_sb[:, fc, cs],
                            start=(fc == 0), stop=(fc == KT - 1))
                    nc.vector.scalar_tensor_tensor(
                        out=out1[:, m, cs], in0=pp[:], scalar=0.0,
                        in1=xo_sb[:, m, cs], op0=ALU.add, op1=ALU.add)

            # ---- LN2 (own 512 tokens) -> h2T (fp8) ----
            ssum2 = sb.tile([P, 4], f32)
            sqs2 = sb.tile([P, 4], f32)
            mu2 = sb.tile([P, 4], f32)
            rstd2 = sb.tile([P, 4], f32)
            nvar2 = sb.tile([P, 4], f32)
            for m in range(4):
                s = slice(m, m + 1)
                nc.vector.tensor_reduce(ssum2[:, s], out1[:, m, :], AX.X,
                                        ALU.add)
                sqo = st.tile([P, C], bf16, tag="sq", bufs=2,
                              name=f"sqo2_{m}")
                nc.scalar.activation(sqo[:], out1[:, m, :], ACT_F.Square,
                                     accum_out=sqs2[:, s])
            nc.vector.tensor_scalar(
                out=mu2[:], in0=ssum2[:], scalar1=1.0 / C,
                scalar2=None, op0=ALU.mult)
            nc.vector.tensor_tensor(out=nvar2[:], in0=mu2[:],
                                    in1=mu2[:], op=ALU.mult)
            nc.vector.scalar_tensor_tensor(
                out=nvar2[:], in0=sqs2[:], scalar=1.0 / C,
                in1=nvar2[:], op0=ALU.mult, op1=ALU.subtract)
            nc.vector.tensor_scalar(
                out=nvar2[:], in0=nvar2[:], scalar1=EPS,
                scalar2=None, op0=ALU.add)
            nc.vector.reciprocal(nvar2[:], nvar2[:])
            nc.scalar.sqrt(rstd2[:], nvar2[:])
            for m in range(4):
                s = slice(m, m + 1)
                h2c = st.tile([P, C], bf16, tag="h", bufs=2,
                              name=f"h2c{m}")
                nc.vector.tensor_scalar(
                    out=h2c[:], in0=out1[:, m, :], scalar1=mu2[:, s],
                    scalar2=rstd2[:, s], op0=ALU.subtract, op1=ALU.mult)
                for g in range(2):
                    tp = ps.tile([P, 4, P], bf16, tag="tp", bufs=1,
                                 name=f"tph2_{m}_{g}")
                    for k in range(4):
                        kt = g * 4 + k
                        nc.tensor.transpose(tp[:, k, :],
                                            h2c[:, kt * P:(kt + 1) * P],
                                            id_bf[:])
                    nc.vector.tensor_copy(
                        out=h2T[:, g * 4:(g + 1) * 4, m * P:(m + 1) * P],
                        in_=tp[:])

            # ---- FFN1: ff1T = relu(W1.T h2T + b1) ----
            ff1T = sb.tile([P, FMT, TOWN], bf16, tag="TA", name="ff1T")
            for mt in range(FMT):
                w1s = st.tile([P, KT, P], bf16, tag="w1", name=f"w1s{mt}")
                nc.sync.dma_start(w1s[:], w1_ext[mt])
                pp = ps.tile([P, TOWN], f32, tag="big", bufs=3,
                             name=f"ff1{mt}")
                for kt in range(KT):
                    nc.tensor.matmul(pp[:], w1s[:, kt, :], h2T[:, kt, :],
                                     start=(kt == 0), stop=(kt == KT - 1))
                nc.scalar.activation(ff1T[:, mt, :], pp[:], ACT_F.Relu,
                                     bias=b1_sb[:, mt:mt + 1])

            # ---- out1T (+b2), cout-major residual-2 ----
            out1T = sb.tile([P, KT, TOWN], f32, tag="TK0", name="out1T")
            for cc in range(KT):
                tp = ps.tile([P, 4, P], f32, tag="tp", bufs=1,
                             name=f"tpo{cc}")
                for m in range(4):
                    nc.tensor.transpose(tp[:, m, :],
                                        out1[:, m, cc * P:(cc + 1) * P],
                                        id_f32[:])
                nc.vector.tensor_scalar(
                    out=out1T[:, cc, :],
                    in0=tp[:].rearrange("p m t -> p (m t)"),
                    scalar1=b2_sb[:, cc:cc + 1], scalar2=None, op0=ALU.add)

            # ---- FFN2: outT = ff1T.T@W2 + (out1T + b2) ----
            for cc in range(KT):
                w2h = []
                for hh in range(4):
                    w2s = st.tile([P, FMT // 4, P], bf16,
                                  tag=f"w2{hh % 2}", bufs=1,
                                  name=f"w2s{cc}_{hh}")
                    nc.sync.dma_start(
                        w2s[:], w2_ext[cc, :, hh * (FMT // 4):
                                       (hh + 1) * (FMT // 4), :])
                    w2h.append(w2s)
                pp = ps.tile([P, TOWN], f32, tag="big", bufs=3,
                             name=f"ff2{cc}")
                for kt in range(FMT):
                    nc.tensor.matmul(pp[:], w2h[kt // 8][:, kt % 8, :],
                                     ff1T[:, kt, :],
                                     start=(kt == 0), stop=(kt == FMT - 1))
                ob = st.tile([P, TOWN], f32, tag="ev", bufs=2,
                             name=f"ob{cc}")
                nc.vector.scalar_tensor_tensor(
                    out=ob[:], in0=pp[:], scalar=0.0,
                    in1=out1T[:, cc, :], op0=ALU.add, op1=ALU.add)
                nc.sync.dma_start(outT_ext[cc * P:(cc + 1) * P, :], ob[:])


_NC_CACHE = None


def _get_nc():
    global _NC_CACHE
    if _NC_CACHE is None:
        _NC_CACHE = build()
    return _NC_CACHE


def shard_inputs(x, Wq, Wk, Wv, Wproj, bproj, W1, b1, W2, b2,
                 ln1_w, ln1_b, ln2_w, ln2_b):
    bf = mybir.dt.np(bf16)
    f8n = mybir.dt.np(f8)
    x = np.asarray(x, np.float32)
    # fold LN1 gamma into Wq/Wk/Wv rows; LN2 gamma into W1 rows
    Wqf = (ln1_w[:, None] * Wq).astype(np.float32)
    Wkf = (ln1_w[:, None] * Wk).astype(np.float32)
    Wvf = (ln1_w[:, None] * Wv).astype(np.float32)
    W1f = (ln2_w[:, None] * W1).astype(np.float32)
    bqf = ln1_b @ Wq                       # query bias (kept)
    bvf = ln1_b @ Wv                       # value bias -> folds via Wproj
    b1f = (ln2_b @ W1 + b1).astype(np.float32)
    # residual-1 base addend: bproj + (value-bias term through proj)
    res_add = (bproj + bvf @ Wproj).astype(np.float32)

    xb = np.ascontiguousarray(x).astype(bf)
    # pre-arranged layouts: [kp, kt, cols]
    wp_b = np.ascontiguousarray(
        Wproj.reshape(KT, P, C).transpose(1, 0, 2)).astype(bf)
    w1_8 = np.ascontiguousarray(
        W1f.reshape(KT, P, FMT, P).transpose(2, 1, 0, 3)).astype(bf)
    w2_8 = np.ascontiguousarray(
        np.asarray(W2, np.float32).reshape(FMT, P, KT, P)
        .transpose(2, 1, 0, 3)).astype(bf)
    b1_r = np.ascontiguousarray(b1f.reshape(FMT, P).T, dtype=np.float32)
    b2_r = np.ascontiguousarray(
        np.asarray(b2, np.float32).reshape(KT, P).T, dtype=np.float32)

    in_maps = []
    for c in range(NCORES):
        b, j = c // 4, c % 4
        hs = slice(P * c, P * (c + 1))
        xo = (x[b, TOWN * j:TOWN * (j + 1)] + res_add).astype(np.float32)
        in_maps.append({
            "xb": xb,
            "xo": np.ascontiguousarray(
                xo.reshape(4, P, C).transpose(1, 0, 2)),
            "wq": np.ascontiguousarray(
                Wqf[:, hs].reshape(KT, P, P).transpose(1, 0, 2)).astype(bf),
            "wk": np.ascontiguousarray(
                Wkf[:, hs].reshape(KT, P, P).transpose(1, 0, 2)).astype(bf),
            "wv": np.ascontiguousarray(
                Wvf[:, hs].reshape(KT, P, P).transpose(1, 0, 2)).astype(bf),
            "wp": wp_b,
            "w1": w1_8,
            "w2": w2_8,
            "bq": np.ascontiguousarray(bqf[hs, None], dtype=np.float32),
            "b1": b1_r,
            "b2": b2_r,
        })
    return in_maps


def assemble(results):
    out = np.empty((2, T, C), np.float32)
    for c in range(NCORES):
        b, j = c // 4, c % 4
        out[b, TOWN * j:TOWN * (j + 1)] = results[c]["outT"].T
    return out


def kernel(**inputs):
    nc = _get_nc()
    in_maps = shard_inputs(**{k: np.asarray(v) for k, v in inputs.items()})
    res = run_bass_kernel_spmd(nc, in_maps, list(range(NCORES)))
    return assemble(res.results)


# revision 53
# speedup vs baseline: 1.2086x; 1.2086x over previous
"""Transformer block (pre-LN attention + FFN) on 8 TRN2 NeuronCores — v3.

Sharding (core c of 8): attention heads {2c, 2c+1} for BOTH batches;
own global token block c (batch c//4, tokens [512*(c%4), +512)) for
proj/LN2/FFN/residual/output.

  - LN1 replicated per batch on every core (no AllGather); batch 1's LN and
    QKV are emitted interleaved into batch 0's attention so every engine
    queue stays busy.
  - One 8-core AllToAll (fp8, split into two token-halves, CC path
    pre-warmed by a dummy collective) moves transposed attention features;
    proj/LN2/FFN run fully local (no ReduceScatter).
  - All weights host-pre-cast and PRE-ARRANGED into the on-chip layouts so
    every DMA is a cheap contiguous descriptor.
  - FFN runs in fp8 (weights host-scaled x16) with DoubleRow perf mode.
  - LN stats are grouped (4 chunks) to minimize ACT table reloads.
  - Output produced transposed ([C, 512] per core), untransposed on host.
"""

import numpy as np

import concourse.bass as bass
import concourse.mybir as mybir
import concourse.tile as tile
from concourse import bacc
from concourse.bass_utils import run_bass_kernel_spmd
from concourse.masks import make_identity

P = 128
C = 1024          # n_embd
KT = C // P       # 8 c-tiles
T = 2048          # tokens per batch
NTC = T // P      # 16 token chunks per batch
TOWN = 512        # own tokens per core
D = 64            # head dim
FF = 4096
FMT = FF // P     # 32 ffn m-tiles
CH = 256          # attention query chunk
QC = T // CH      # 8 chunks
EPS = 1e-5
SCALE = 1.0 / 32.0  # C ** -0.5
W8S = 16.0          # host-side fp8 weight scale for W1/W2
GROUP8 = [[0, 1, 2, 3, 4, 5, 6, 7]]
NCORES = 8

f32 = mybir.dt.float32
bf16 = mybir.dt.bfloat16
f8 = mybir.dt.float8e4
AX = mybir.AxisListType
ALU = mybir.AluOpType
ACT_F = mybir.ActivationFunctionType
DR = mybir.MatmulPerfMode.DoubleRow


def build():
    nc = bacc.Bacc("TRN2", target_bir_lowering=False, debug=False,
                   num_devices=NCORES)
    _build_graph(nc)
    nc.compile()
    return nc


def _build_graph(nc):
    xb_ext = nc.dram_tensor("xb", [2, T, C], bf16, kind="ExternalInput").ap()
    xo_ext = nc.dram_tensor("xo", [P, 4, C], f32, kind="ExternalInput").ap()
    wq_ext = nc.dram_tensor("wq", [P, KT, P], bf16, kind="ExternalInput").ap()
    wk_ext = nc.dram_tensor("wk", [P, KT, P], bf16, kind="ExternalInput").ap()
    wv_ext = nc.dram_tensor("wv", [P, KT, P], bf16, kind="ExternalInput").ap()
    wp_ext = nc.dram_tensor("wp", [P, KT, C], bf16, kind="ExternalInput").ap()
    w1_ext = nc.dram_tensor("w1", [FMT, P, KT, P], bf16,
                            kind="ExternalInput").ap()
    w2_ext = nc.dram_tensor("w2", [KT, P, FMT, P], bf16,
                            kind="ExternalInput").ap()
    bq_ext = nc.dram_tensor("bq", [P, 1], f32, kind="ExternalInput").ap()
    b1_ext = nc.dram_tensor("b1", [P, FMT], f32, kind="ExternalInput").ap()
    b2_ext = nc.dram_tensor("b2", [P, KT], f32, kind="ExternalInput").ap()
    outT_ext = nc.dram_tensor("outT", [C, TOWN], f32,
                              kind="ExternalOutput").ap()

    with tile.TileContext(nc) as tc:
        with (
            tc.tile_pool(name="sb", bufs=1) as sb,
            tc.tile_pool(name="st", bufs=3) as st,
            tc.tile_pool(name="ps", bufs=1, space="PSUM") as ps,
            tc.tile_pool(name="dram", bufs=1, space="DRAM") as dram,
        ):
            # ---- constants ----
            id_bf = sb.tile([P, P], bf16)
            make_identity(nc, id_bf[:])
            id_f32 = sb.tile([P, P], f32)
            make_identity(nc, id_f32[:])
            # causal mask for diagonal blocks, layout [key_p, hl, query]
            # per key-shift sh: keep where key (128*sh + p) <= query y
            mask = sb.tile([P, 2, 2, CH], bf16)
            nc.gpsimd.memset(mask[:], 1.0)
            nc.gpsimd.affine_select(
                out=mask[:], in_=mask[:], compare_op=ALU.is_ge, fill=0.0,
                base=0, pattern=[[-P, 2], [0, 2], [1, CH]],
                channel_multiplier=-1)

            # ---- CC warmup: tiny AllToAll so the real ones start fast.
            # GpSimd has nothing else queued before the real triggers, so
            # this can block its queue harmlessly while absorbing skew.
            warm_in = dram.tile([NCORES * P, 64], f8, name="warm_in")
            warm_out = dram.tile([NCORES * P, 64], f8, name="warm_out")
            warm_sb = sb.tile([P, 64], f8)
            nc.vector.memset(warm_sb[:], 0.0)
            for i in range(NCORES):
                nc.sync.dma_start(warm_in[i * P:(i + 1) * P, :], warm_sb[:])
            nc.gpsimd.collective_compute(
                "AllToAll", ALU.bypass, ins=[warm_in.opt()],
                outs=[warm_out.opt()], replica_groups=GROUP8)

            # LN1 per-token stats, one column per (batch, token chunk)
            ssum = sb.tile([P, 2 * NTC], f32)
            sqs = sb.tile([P, 2 * NTC], f32)
            mu = sb.tile([P, 2 * NTC], f32)
            rstd = sb.tile([P, 2 * NTC], f32)
            nvar = sb.tile([P, 2 * NTC], f32)

            xbc_tiles = {}

            def ln1_stats_chunk(b, tci):
                """DMA chunk, accumulate sum and sum-of-squares.
                Row-sums via STT-with-accum (16-bit in/out)."""
                s = slice(b * NTC + tci, b * NTC + tci + 1)
                xbc = st.tile([P, C], bf16, tag="xb", bufs=8,
                              name=f"xbc{b}_{tci}")
                xbc_tiles[(b, tci)] = xbc
                nc.sync.dma_start(xbc[:], xb_ext[b, tci * P:(tci + 1) * P, :])
                so = st.tile([P, C], bf16, tag="sq", bufs=2,
                             name=f"so{b}_{tci}")
                nc.vector.scalar_tensor_tensor(
                    out=so[:], in0=xbc[:], scalar=0.0, in1=xbc[:],
                    op0=ALU.add, op1=ALU.bypass, accum_out=ssum[:, s])
                if b == 0:
                    sqo = st.tile([P, C], bf16, tag="sq", bufs=2,
                                  name=f"sqo{b}_{tci}")
                    nc.scalar.activation(sqo[:], xbc[:], ACT_F.Square,
                                         accum_out=sqs[:, s])
                else:
                    sqo = st.tile([P, C], bf16, tag="sq", bufs=2,
                                  name=f"sqo{b}_{tci}")
                    nc.vector.scalar_tensor_tensor(
                        out=sqo[:], in0=xbc[:], scalar=1.0, in1=xbc[:],
                        op0=ALU.mult, op1=ALU.mult, accum_out=sqs[:, s])

            def ln_group_stats(sl):
                """Batched stats for a group of chunk columns sl."""
                nc.vector.tensor_scalar(
                    out=mu[:, sl], in0=ssum[:, sl], scalar1=1.0 / C,
                    scalar2=None, op0=ALU.mult)
                nc.vector.tensor_tensor(out=nvar[:, sl], in0=mu[:, sl],
                                        in1=mu[:, sl], op=ALU.mult)
                nc.vector.scalar_tensor_tensor(
                    out=nvar[:, sl], in0=sqs[:, sl], scalar=1.0 / C,
                    in1=nvar[:, sl], op0=ALU.mult, op1=ALU.subtract)
                nc.vector.tensor_scalar(
                    out=nvar[:, sl], in0=nvar[:, sl], scalar1=EPS,
                    scalar2=None, op0=ALU.add)
                nc.vector.reciprocal(nvar[:, sl], nvar[:, sl])
                nc.scalar.sqrt(rstd[:, sl], nvar[:, sl])

            def ln1_apply_chunk(b, tci, hT):
                """Normalize chunk and transpose into hT (packed evac)."""
                s = slice(b * NTC + tci, b * NTC + tci + 1)
                xbc = xbc_tiles.pop((b, tci))
                hc = st.tile([P, C], bf16, tag="h", bufs=2,
                             name=f"hc{b}_{tci}")
                nc.vector.tensor_scalar(
                    out=hc[:], in0=xbc[:], scalar1=mu[:, s],
                    scalar2=rstd[:, s], op0=ALU.subtract, op1=ALU.mult)
                tp = ps.tile([P, KT, P], bf16, tag="tp", bufs=1,
                             name=f"tph{b}_{tci}")
                for kt in range(KT):
                    nc.tensor.transpose(tp[:, kt, :],
                                        hc[:, kt * P:(kt + 1) * P],
                                        id_bf[:])
                nc.vector.tensor_copy(
                    out=hT[:, :, tci * P:(tci + 1) * P], in_=tp[:])

            def qkv_k(b, hT, kT, w_sb, th):
                pp = ps.tile([P, TOWN], f32, tag="big", bufs=3,
                             name=f"k{b}_{th}")
                for kt in range(KT):
                    nc.tensor.matmul(
                        pp[:], w_sb[:, kt, :],
                        hT[:, kt, th * TOWN:(th + 1) * TOWN],
                        start=(kt == 0), stop=(kt == KT - 1))
                ts_ = slice(th * TOWN, (th + 1) * TOWN)
                nc.vector.tensor_copy(out=kT[0:D, 0, ts_], in_=pp[0:D, :])
                nc.vector.tensor_copy(out=kT[D:P, 1, ts_], in_=pp[D:P, :])

            def qkv_q(b, hT, qT, w_sb, bq_sb, th):
                pp = ps.tile([P, TOWN], f32, tag="big", bufs=3,
                             name=f"q{b}_{th}")
                for kt in range(KT):
                    nc.tensor.matmul(
                        pp[:], w_sb[:, kt, :],
                        hT[:, kt, th * TOWN:(th + 1) * TOWN],
                        start=(kt == 0), stop=(kt == KT - 1))
                nc.vector.tensor_scalar(
                    out=qT[:, th * TOWN:(th + 1) * TOWN], in0=pp[:],
                    scalar1=bq_sb[:], scalar2=None, op0=ALU.add)

            def qkv_v(b, hT, v_aug, w_sb, tci):
                pp = ps.tile([P, P], f32, tag="tp", bufs=1,
                             name=f"v{b}_{tci}")
                for kt in range(KT):
                    nc.tensor.matmul(
                        pp[:], hT[:, kt, tci * P:(tci + 1) * P],
                        w_sb[:, kt, :],
                        start=(kt == 0), stop=(kt == KT - 1))
                nc.vector.tensor_copy(
                    out=v_aug[:, tci, :, 0:D],
                    in_=pp[:].rearrange("p (h d) -> p h d", d=D))

            def attn_qc(b, qc, kT, qT, v_aug, attn_sb):
                """Scores+exp for ALL key chunks first (keeps the scalar
                engine's exp stream continuous), then AV matmuls with V as
                the stationary operand (one weight load per key chunk, wide
                moving operand) producing feature-major partials [65, 256]
                that are transposed+normalized at evacuation."""
                if True:
                    aps = [ps.tile([P, D + 1], f32, tag="aps", bufs=4,
                                   name=f"aps{b}_{qc}_{i}")
                           for i in range(4)]
                    pend = []

                    def flush_avs():
                        for kc_, sh_, ex_ in pend:
                            for hl in range(2):
                                for ti in range(2):
                                    nc.tensor.matmul(
                                        aps[hl * 2 + ti][:],
                                        ex_[:, hl, ti * P:(ti + 1) * P],
                                        v_aug[:, 2 * kc_ + sh_, hl, :],
                                        start=(kc_ == 0 and sh_ == 0),
                                        stop=(kc_ == qc and sh_ == 1))
                        pend.clear()

                    for kc in range(qc + 1):
                        for sh in range(2):
                            sc = ps.tile([P, 2, CH], f32, tag="big", bufs=3,
                                         name=f"sc{b}_{qc}_{kc}_{sh}")
                            for hl in range(2):
                                nc.tensor.matmul(
                                    sc[:, hl, :],
                                    kT[:, hl,
                                       kc * CH + sh * P:kc * CH + (sh + 1) * P],
                                    qT[:, qc * CH:(qc + 1) * CH],
                                    start=True, stop=True)
                            ex = st.tile([P, 2, CH], bf16, tag="ex", bufs=9,
                                         name=f"ex{b}_{qc}_{kc}_{sh}")
                            nc.scalar.activation(ex[:], sc[:], ACT_F.Exp,
                                                 bias=0.0, scale=SCALE)
                            if kc == qc:
                                nc.vector.tensor_tensor(
                                    out=ex[:], in0=ex[:], in1=mask[:, sh],
                                    op=ALU.mult)
                            pend.append((kc, sh, ex))
                        if len(pend) >= 8:
                            flush_avs()
                    flush_avs()
                    for ti in range(2):
                        for hl in range(2):
                            rd = st.tile([P, 1], f32, tag="rd", bufs=4,
                                         name=f"rd{b}_{qc}_{ti}_{hl}")
                            nc.vector.reciprocal(rd[:],
                                                 aps[hl * 2 + ti][:, D:D + 1])
                            nc.vector.tensor_scalar(
                                out=attn_sb[:, 2 * qc + ti,
                                            hl * D:(hl + 1) * D],
                                in0=aps[hl * 2 + ti][:, 0:D], scalar1=rd[:],
                                scalar2=None, op0=ALU.mult)

            # ---- persistent tiles ----
            hT_0 = sb.tile([P, KT, T], bf16, tag="TA", name="hT_0")
            kT_0 = sb.tile([P, 2, T], bf16, tag="TK0", name="kT_0")
            qT_0 = sb.tile([P, T], bf16, name="qT_0")
            va_0 = sb.tile([P, NTC, 2, D + 1], bf16, name="va_0")
            hT_1 = sb.tile([P, KT, T], bf16, tag="TA", name="hT_1")
            kT_1 = sb.tile([P, 2, T], bf16, name="kT_1")
            qT_1 = sb.tile([P, T], bf16, name="qT_1")
            va_1 = sb.tile([P, NTC, 2, D + 1], bf16, name="va_1")
            at_0 = sb.tile([P, NTC, P], bf16, name="at_0")
            at_1 = sb.tile([P, NTC, P], bf16, name="at_1")
            afT = sb.tile([P, KT, TOWN], f8, name="afT")
            out1 = sb.tile([P, 4, C], f32, name="out1")
            h2T = sb.tile([P, KT, TOWN], bf16, name="h2T")

            # ---- weights (contiguous, host-prearranged) ----
            wq_sb = sb.tile([P, KT, P], bf16)
            wk_sb = sb.tile([P, KT, P], bf16)
            wv_sb = sb.tile([P, KT, P], bf16)
            bq_sb = sb.tile([P, 1], f32)
            for w_sb, ext in ((wk_sb, wk_ext), (wq_sb, wq_ext),
                              (wv_sb, wv_ext), (bq_sb, bq_ext)):
                nc.sync.dma_start(w_sb[:], ext[:])

            # ---- unified schedule: LN1/QKV/attention for both batches ----
            # Early attention query-chunks are emitted as soon as the K/Q/V
            # slices they read exist, keeping tensor/vector/scalar all busy.
            nc.vector.memset(kT_0[D:P, 0, :], 0.0)
            nc.vector.memset(kT_0[0:D, 1, :], 0.0)
            nc.vector.memset(va_0[:, :, :, D:D + 1], 1.0)

            def stats(b, lo, hi):
                for tci in range(lo, hi):
                    ln1_stats_chunk(b, tci)

            def finish(b, lo, hi):
                ln_group_stats(slice(b * NTC + lo, b * NTC + hi))

            def apply(b, lo, hi, hT):
                for tci in range(lo, hi):
                    ln1_apply_chunk(b, tci, hT)

            def vs(b, hT, va, w_sb, lo, hi):
                for tci in range(lo, hi):
                    qkv_v(b, hT, va, w_sb, tci)

            # a2a input buffers (fp8, asymmetric token split: the first
            # collective pays the CC ramp, so keep it small)
            A2A0 = 128
            a2a_in = [dram.tile([NCORES * P, A2A0], f8, name="a2ain0"),
                      dram.tile([NCORES * P, TOWN - A2A0], f8,
                                name="a2ain1")]
            a2a_out = [dram.tile([NCORES * P, A2A0], f8, name="a2aout0"),
                       dram.tile([NCORES * P, TOWN - A2A0], f8,
                                 name="a2aout1")]

            def a2a_send_block(b, attn_sb, ib):
                # transpose one 512-token block of batch-b attention and
                # stage it; global block i = b*4 + ib -> rows i*128..
                i = b * 4 + ib
                atT = st.tile([P, TOWN], f8, tag="atT", bufs=2, name=f"atT{i}")
                tp = ps.tile([P, 4, P], bf16, tag="tp", bufs=1,
                             name=f"tpa{i}")
                for tt in range(4):
                    nc.tensor.transpose(tp[:, tt, :],
                                        attn_sb[:, ib * 4 + tt, :],
                                        id_bf[:])
                nc.vector.tensor_copy(out=atT[:], in_=tp[:])
                nc.sync.dma_start(a2a_in[0][i * P:(i + 1) * P, :],
                                  atT[:, 0:A2A0])
                nc.sync.dma_start(a2a_in[1][i * P:(i + 1) * P, :],
                                  atT[:, A2A0:TOWN])

            def A0(qc):
                attn_qc(0, qc, kT_0, qT_0, va_0, at_0)

            def A1(qc):
                attn_qc(1, qc, kT_1, qT_1, va_1, at_1)

            # schedule (v4b layout): one attention qc per slot, compact
            # build blocks between; batch-1 build rides attention(0)'s tail
            stats(0, 0, 4)
            finish(0, 0, 4)
            apply(0, 0, 4, hT_0)
            stats(0, 4, 8)
            qkv_k(0, hT_0, kT_0, wk_sb, 0)
            qkv_q(0, hT_0, qT_0, wq_sb, bq_sb, 0)
            vs(0, hT_0, va_0, wv_sb, 0, 2)
            A0(0)
            finish(0, 4, 8)
            apply(0, 4, 8, hT_0)
            qkv_k(0, hT_0, kT_0, wk_sb, 1)
            qkv_q(0, hT_0, qT_0, wq_sb, bq_sb, 1)
            vs(0, hT_0, va_0, wv_sb, 2, 4)
            A0(1)
            stats(0, 8, 16)
            finish(0, 8, 16)
            apply(0, 8, 12, hT_0)
            qkv_k(0, hT_0, kT_0, wk_sb, 2)
            qkv_q(0, hT_0, qT_0, wq_sb, bq_sb, 2)
            vs(0, hT_0, va_0, wv_sb, 4, 8)
            A0(2)
            A0(3)
            apply(0, 12, 16, hT_0)
            qkv_k(0, hT_0, kT_0, wk_sb, 3)
            qkv_q(0, hT_0, qT_0, wq_sb, bq_sb, 3)
            vs(0, hT_0, va_0, wv_sb, 8, 16)

            # remaining weights (needed much later)
            wp_sb = sb.tile([P, KT, C], bf16)
            nc.sync.dma_start(wp_sb[:], wp_ext[:])
            xo_sb = sb.tile([P, 4, C], f32)
            nc.sync.dma_start(xo_sb[:], xo_ext[:])
            b1_sb = sb.tile([P, FMT], f32)
            nc.sync.dma_start(b1_sb[:], b1_ext[:])
            b2_sb = sb.tile([P, KT], f32)
            nc.sync.dma_start(b2_sb[:], b2_ext[:])

            A0(4)
            stats(1, 0, 4)
            A0(5)
            stats(1, 4, 8)
            finish(1, 0, 8)
            apply(1, 0, 4, hT_1)
            A0(6)
            stats(1, 8, 12)
            apply(1, 4, 8, hT_1)
            nc.vector.memset(kT_1[D:P, 0, :], 0.0)
            nc.vector.memset(kT_1[0:D, 1, :], 0.0)
            nc.vector.memset(va_1[:, :, :, D:D + 1], 1.0)
            qkv_k(1, hT_1, kT_1, wk_sb, 0)
            qkv_q(1, hT_1, qT_1, wq_sb, bq_sb, 0)
            A0(7)
            stats(1, 12, 16)
            finish(1, 8, 16)
            apply(1, 8, 12, hT_1)
            qkv_k(1, hT_1, kT_1, wk_sb, 1)
            qkv_q(1, hT_1, qT_1, wq_sb, bq_sb, 1)
            vs(1, hT_1, va_1, wv_sb, 0, 4)
            for ib in range(4):
                a2a_send_block(0, at_0, ib)
            apply(1, 12, 16, hT_1)
            A1(0)
            A1(1)
            qkv_k(1, hT_1, kT_1, wk_sb, 2)
            qkv_q(1, hT_1, qT_1, wq_sb, bq_sb, 2)
            vs(1, hT_1, va_1, wv_sb, 4, 10)
            A1(2)
            A1(3)
            qkv_k(1, hT_1, kT_1, wk_sb, 3)
            qkv_q(1, hT_1, qT_1, wq_sb, bq_sb, 3)
            vs(1, hT_1, va_1, wv_sb, 10, 16)
            A1(4)
            A1(5)
            A1(6)
            A1(7)
            for ib in range(4):
                a2a_send_block(1, at_1, ib)
            for h in range(2):
                nc.gpsimd.collective_compute(
                    "AllToAll", ALU.bypass, ins=[a2a_in[h].opt()],
                    outs=[a2a_out[h].opt()], replica_groups=GROUP8)
            for s_i in range(NCORES):
                nc.sync.dma_start(afT[:, s_i, 0:A2A0],
                                  a2a_out[0][s_i * P:(s_i + 1) * P, :])
            for s_i in range(NCORES):
                nc.sync.dma_start(afT[:, s_i, A2A0:TOWN],
                                  a2a_out[1][s_i * P:(s_i + 1) * P, :])

            # ---- proj + residual (own tokens, token-major) ----
            for m in range(4):
                for nh in range(2):
                    cs = slice(nh * TOWN, (nh + 1) * TOWN)
                    pp = ps.tile([P, TOWN], f32, tag="big", bufs=3,
                                 name=f"proj{m}_{nh}")
                    for fc in range(KT):
                        nc.tensor.matmul(
                            pp[:], afT[:, fc, m * P:(m + 1) * P],
                            wp_sb[:, fc, cs],
                            start=(fc == 0), stop=(fc == KT - 1))
                    nc.vector.scalar_tensor_tensor(
                        out=out1[:, m, cs], in0=pp[:], scalar=0.0,
                        in1=xo_sb[:, m, cs], op0=ALU.add, op1=ALU.add)

            # ---- LN2 (own 512 tokens) -> h2T (fp8) ----
            ssum2 = sb.tile([P, 4], f32)
            sqs2 = sb.tile([P, 4], f32)
            mu2 = sb.tile([P, 4], f32)
            rstd2 = sb.tile([P, 4], f32)
            nvar2 = sb.tile([P, 4], f32)
            for m in range(4):
                s = slice(m, m + 1)
                nc.vector.tensor_reduce(ssum2[:, s], out1[:, m, :], AX.X,
                                        ALU.add)
                sqo = st.tile([P, C], bf16, tag="sq", bufs=2,
                              name=f"sqo2_{m}")
                nc.scalar.activation(sqo[:], out1[:, m, :], ACT_F.Square,
                                     accum_out=sqs2[:, s])
            nc.vector.tensor_scalar(
                out=mu2[:], in0=ssum2[:], scalar1=1.0 / C,
                scalar2=None, op0=ALU.mult)
            nc.vector.tensor_tensor(out=nvar2[:], in0=mu2[:],
                                    in1=mu2[:], op=ALU.mult)
            nc.vector.scalar_tensor_tensor(
                out=nvar2[:], in0=sqs2[:], scalar=1.0 / C,
                in1=nvar2[:], op0=ALU.mult, op1=ALU.subtract)
            nc.vector.tensor_scalar(
                out=nvar2[:], in0=nvar2[:], scalar1=EPS,
                scalar2=None, op0=ALU.add)
            nc.vector.reciprocal(nvar2[:], nvar2[:])
            nc.scalar.sqrt(rstd2[:], nvar2[:])
            for m in range(4):
                s = slice(m, m + 1)
                h2c = st.tile([P, C], bf16, tag="h", bufs=2,
                              name=f"h2c{m}")
                nc.vector.tensor_scalar(
                    out=h2c[:], in0=out1[:, m, :], scalar1=mu2[:, s],
                    scalar2=rstd2[:, s], op0=ALU.subtract, op1=ALU.mult)
                for g in range(2):
                    tp = ps.tile([P, 4, P], bf16, tag="tp", bufs=1,
                                 name=f"tph2_{m}_{g}")
                    for k in range(4):
                        kt = g * 4 + k
                        nc.tensor.transpose(tp[:, k, :],
                                            h2c[:, kt * P:(kt + 1) * P],
                                            id_bf[:])
                    nc.vector.tensor_copy(
                        out=h2T[:, g * 4:(g + 1) * 4, m * P:(m + 1) * P],
                        in_=tp[:])

            # ---- FFN1: ff1T = relu(W1.T h2T + b1) ----
            ff1T = sb.tile([P, FMT, TOWN], bf16, tag="TA", name="ff1T")
            for mt in range(FMT):
                w1s = st.tile([P, KT, P], bf16, tag="w1", name=f"w1s{mt}")
                nc.sync.dma_start(w1s[:], w1_ext[mt])
                pp = ps.tile([P, TOWN], f32, tag="big", bufs=3,
                             name=f"ff1{mt}")
                for kt in range(KT):
                    nc.tensor.matmul(pp[:], w1s[:, kt, :], h2T[:, kt, :],
                                     start=(kt == 0), stop=(kt == KT - 1))
                nc.scalar.activation(ff1T[:, mt, :], pp[:], ACT_F.Relu,
                                     bias=b1_sb[:, mt:mt + 1])

            # ---- out1T (+b2), cout-major residual-2 ----
            out1T = sb.tile([P, KT, TOWN], f32, tag="TK0", name="out1T")
            for cc in range(KT):
                tp = ps.tile([P, 4, P], f32, tag="tp", bufs=1,
                             name=f"tpo{cc}")
                for m in range(4):
                    nc.tensor.transpose(tp[:, m, :],
                                        out1[:, m, cc * P:(cc + 1) * P],
                                        id_f32[:])
                nc.vector.tensor_scalar(
                    out=out1T[:, cc, :],
                    in0=tp[:].rearrange("p m t -> p (m t)"),
                    scalar1=b2_sb[:, cc:cc + 1], scalar2=None, op0=ALU.add)

            # ---- FFN2: outT = ff1T.T@W2 + (out1T + b2) ----
            for cc in range(KT):
                w2h = []
                for hh in range(4):
                    w2s = st.tile([P, FMT // 4, P], bf16,
                                  tag=f"w2{hh % 2}", bufs=1,
                                  name=f"w2s{cc}_{hh}")
                    nc.sync.dma_start(
                        w2s[:], w2_ext[cc, :, hh * (FMT // 4):
                                       (hh + 1) * (FMT // 4), :])
                    w2h.append(w2s)
                pp = ps.tile([P, TOWN], f32, tag="big", bufs=3,
                             name=f"ff2{cc}")
                for kt in range(FMT):
                    nc.tensor.matmul(pp[:], w2h[kt // 8][:, kt % 8, :],
                                     ff1T[:, kt, :],
                                     start=(kt == 0), stop=(kt == FMT - 1))
                ob = st.tile([P, TOWN], f32, tag="ev", bufs=2,
                             name=f"ob{cc}")
                nc.vector.scalar_tensor_tensor(
                    out=ob[:], in0=pp[:], scalar=0.0,
                    in1=out1T[:, cc, :], op0=ALU.add, op1=ALU.add)
                nc.sync.dma_start(outT_ext[cc * P:(cc + 1) * P, :], ob[:])


_NC_CACHE = None


def _get_nc():
    global _NC_CACHE
    if _NC_CACHE is None:
        _NC_CACHE = build()
    return _NC_CACHE


def shard_inputs(x, Wq, Wk, Wv, Wproj, bproj, W1, b1, W2, b2,
                 ln1_w, ln1_b, ln2_w, ln2_b):
    bf = mybir.dt.np(bf16)
    f8n = mybir.dt.np(f8)
    x = np.asarray(x, np.float32)
    # fold LN1 gamma into Wq/Wk/Wv rows; LN2 gamma into W1 rows
    Wqf = (ln1_w[:, None] * Wq).astype(np.float32)
    Wkf = (ln1_w[:, None] * Wk).astype(np.float32)
    Wvf = (ln1_w[:, None] * Wv).astype(np.float32)
    W1f = (ln2_w[:, None] * W1).astype(np.float32)
    bqf = ln1_b @ Wq                       # query bias (kept)
    bvf = ln1_b @ Wv                       # value bias -> folds via Wproj
    b1f = (ln2_b @ W1 + b1).astype(np.float32)
    # residual-1 base addend: bproj + (value-bias term through proj)
    res_add = (bproj + bvf @ Wproj).astype(np.float32)

    xb = np.ascontiguousarray(x).astype(bf)
    # pre-arranged layouts: [kp, kt, cols]
    wp_b = np.ascontiguousarray(
        Wproj.reshape(KT, P, C).transpose(1, 0, 2)).astype(bf)
    w1_8 = np.ascontiguousarray(
        W1f.reshape(KT, P, FMT, P).transpose(2, 1, 0, 3)).astype(bf)
    w2_8 = np.ascontiguousarray(
        np.asarray(W2, np.float32).reshape(FMT, P, KT, P)
        .transpose(2, 1, 0, 3)).astype(bf)
    b1_r = np.ascontiguousarray(b1f.reshape(FMT, P).T, dtype=np.float32)
    b2_r = np.ascontiguousarray(
        np.asarray(b2, np.float32).reshape(KT, P).T, dtype=np.float32)

    in_maps = []
    for c in range(NCORES):
        b, j = c // 4, c % 4
        hs = slice(P * c, P * (c + 1))
        xo = (x[b, TOWN * j:TOWN * (j + 1)] + res_add).astype(np.float32)
        in_maps.append({
            "xb": xb,
            "xo": np.ascontiguousarray(
                xo.reshape(4, P, C).transpose(1, 0, 2)),
            "wq": np.ascontiguousarray(
                Wqf[:, hs].reshape(KT, P, P).transpose(1, 0, 2)).astype(bf),
            "wk": np.ascontiguousarray(
                Wkf[:, hs].reshape(KT, P, P).transpose(1, 0, 2)).astype(bf),
            "wv": np.ascontiguousarray(
                Wvf[:, hs].reshape(KT, P, P).transpose(1, 0, 2)).astype(bf),
            "wp": wp_b,
            "w1": w1_8,
            "w2": w2_8,
            "bq": np.ascontiguousarray(bqf[hs, None], dtype=np.float32),
            "b1": b1_r,
            "b2": b2_r,
        })
    return in_maps


def assemble(results):
    out = np.empty((2, T, C), np.float32)
    for c in range(NCORES):
        b, j = c // 4, c % 4
        out[b, TOWN * j:TOWN * (j + 1)] = results[c]["outT"].T
    return out


def kernel(**inputs):
    nc = _get_nc()
    in_maps = shard_inputs(**{k: np.asarray(v) for k, v in inputs.items()})
    res = run_bass_kernel_spmd(nc, in_maps, list(range(NCORES)))
    return assemble(res.results)


# revision 54
# speedup vs baseline: 1.2385x; 1.0247x over previous
"""Transformer block (pre-LN attention + FFN) on 8 TRN2 NeuronCores — v3.

Sharding (core c of 8): attention heads {2c, 2c+1} for BOTH batches;
own global token block c (batch c//4, tokens [512*(c%4), +512)) for
proj/LN2/FFN/residual/output.

  - LN1 replicated per batch on every core (no AllGather); batch 1's LN and
    QKV are emitted interleaved into batch 0's attention so every engine
    queue stays busy.
  - One 8-core AllToAll (fp8, split into two token-halves, CC path
    pre-warmed by a dummy collective) moves transposed attention features;
    proj/LN2/FFN run fully local (no ReduceScatter).
  - All weights host-pre-cast and PRE-ARRANGED into the on-chip layouts so
    every DMA is a cheap contiguous descriptor.
  - FFN runs in fp8 (weights host-scaled x16) with DoubleRow perf mode.
  - LN stats are grouped (4 chunks) to minimize ACT table reloads.
  - Output produced transposed ([C, 512] per core), untransposed on host.
"""

import numpy as np

import concourse.bass as bass
import concourse.mybir as mybir
import concourse.tile as tile
from concourse import bacc
from concourse.bass_utils import run_bass_kernel_spmd
from concourse.masks import make_identity

P = 128
C = 1024          # n_embd
KT = C // P       # 8 c-tiles
T = 2048          # tokens per batch
NTC = T // P      # 16 token chunks per batch
TOWN = 512        # own tokens per core
D = 64            # head dim
FF = 4096
FMT = FF // P     # 32 ffn m-tiles
CH = 256          # attention query chunk
QC = T // CH      # 8 chunks
EPS = 1e-5
SCALE = 1.0 / 32.0  # C ** -0.5
W8S = 16.0          # host-side fp8 weight scale for W1/W2
GROUP8 = [[0, 1, 2, 3, 4, 5, 6, 7]]
NCORES = 8

f32 = mybir.dt.float32
bf16 = mybir.dt.bfloat16
f8 = mybir.dt.float8e4
AX = mybir.AxisListType
ALU = mybir.AluOpType
ACT_F = mybir.ActivationFunctionType
DR = mybir.MatmulPerfMode.DoubleRow


def build():
    nc = bacc.Bacc("TRN2", target_bir_lowering=False, debug=False,
                   num_devices=NCORES)
    _build_graph(nc)
    nc.compile()
    return nc


def _build_graph(nc):
    xb_ext = nc.dram_tensor("xb", [2, T, C], bf16, kind="ExternalInput").ap()
    xo_ext = nc.dram_tensor("xo", [P, 4, C], f32, kind="ExternalInput").ap()
    wq_ext = nc.dram_tensor("wq", [P, KT, P], bf16, kind="ExternalInput").ap()
    wk_ext = nc.dram_tensor("wk", [P, KT, P], bf16, kind="ExternalInput").ap()
    wv_ext = nc.dram_tensor("wv", [P, KT, P], bf16, kind="ExternalInput").ap()
    wp_ext = nc.dram_tensor("wp", [P, KT, C], bf16, kind="ExternalInput").ap()
    w1_ext = nc.dram_tensor("w1", [FMT, P, KT, P], bf16,
                            kind="ExternalInput").ap()
    w2_ext = nc.dram_tensor("w2", [KT, P, FMT, P], bf16,
                            kind="ExternalInput").ap()
    bq_ext = nc.dram_tensor("bq", [P, 1], f32, kind="ExternalInput").ap()
    b1_ext = nc.dram_tensor("b1", [P, FMT], f32, kind="ExternalInput").ap()
    b2_ext = nc.dram_tensor("b2", [P, KT], f32, kind="ExternalInput").ap()
    outT_ext = nc.dram_tensor("outT", [C, TOWN], f32,
                              kind="ExternalOutput").ap()

    with tile.TileContext(nc) as tc:
        with (
            tc.tile_pool(name="sb", bufs=1) as sb,
            tc.tile_pool(name="st", bufs=3) as st,
            tc.tile_pool(name="ps", bufs=1, space="PSUM") as ps,
            tc.tile_pool(name="dram", bufs=1, space="DRAM") as dram,
        ):
            # ---- constants ----
            id_bf = sb.tile([P, P], bf16)
            make_identity(nc, id_bf[:])
            id_f32 = sb.tile([P, P], f32)
            make_identity(nc, id_f32[:])
            # causal mask for diagonal blocks, layout [key_p, hl, query]
            # per key-shift sh: keep where key (128*sh + p) <= query y
            mask = sb.tile([P, 2, 2, CH], bf16)
            nc.gpsimd.memset(mask[:], 1.0)
            nc.gpsimd.affine_select(
                out=mask[:], in_=mask[:], compare_op=ALU.is_ge, fill=0.0,
                base=0, pattern=[[-P, 2], [0, 2], [1, CH]],
                channel_multiplier=-1)

            # ---- CC warmup: tiny AllToAll so the real ones start fast.
            # GpSimd has nothing else queued before the real triggers, so
            # this can block its queue harmlessly while absorbing skew.
            warm_in = dram.tile([NCORES * P, 64], f8, name="warm_in")
            warm_out = dram.tile([NCORES * P, 64], f8, name="warm_out")
            warm_sb = sb.tile([P, 64], f8)
            nc.vector.memset(warm_sb[:], 0.0)
            for i in range(NCORES):
                nc.sync.dma_start(warm_in[i * P:(i + 1) * P, :], warm_sb[:])
            nc.gpsimd.collective_compute(
                "AllToAll", ALU.bypass, ins=[warm_in.opt()],
                outs=[warm_out.opt()], replica_groups=GROUP8)

            # LN1 per-token stats, one column per (batch, token chunk)
            ssum = sb.tile([P, 2 * NTC], f32)
            sqs = sb.tile([P, 2 * NTC], f32)
            mu = sb.tile([P, 2 * NTC], f32)
            rstd = sb.tile([P, 2 * NTC], f32)
            nvar = sb.tile([P, 2 * NTC], f32)

            xbc_tiles = {}

            def ln1_stats_chunk(b, tci):
                """DMA chunk, accumulate sum and sum-of-squares.
                Row-sums via STT-with-accum (16-bit in/out)."""
                s = slice(b * NTC + tci, b * NTC + tci + 1)
                xbc = st.tile([P, C], bf16, tag="xb", bufs=8,
                              name=f"xbc{b}_{tci}")
                xbc_tiles[(b, tci)] = xbc
                nc.sync.dma_start(xbc[:], xb_ext[b, tci * P:(tci + 1) * P, :])
                so = st.tile([P, C], bf16, tag="sq", bufs=2,
                             name=f"so{b}_{tci}")
                nc.vector.scalar_tensor_tensor(
                    out=so[:], in0=xbc[:], scalar=0.0, in1=xbc[:],
                    op0=ALU.add, op1=ALU.bypass, accum_out=ssum[:, s])
                if b == 0:
                    sqo = st.tile([P, C], bf16, tag="sq", bufs=2,
                                  name=f"sqo{b}_{tci}")
                    nc.scalar.activation(sqo[:], xbc[:], ACT_F.Square,
                                         accum_out=sqs[:, s])
                else:
                    sqo = st.tile([P, C], bf16, tag="sq", bufs=2,
                                  name=f"sqo{b}_{tci}")
                    nc.vector.scalar_tensor_tensor(
                        out=sqo[:], in0=xbc[:], scalar=1.0, in1=xbc[:],
                        op0=ALU.mult, op1=ALU.mult, accum_out=sqs[:, s])

            def ln_group_stats(sl):
                """Batched stats for a group of chunk columns sl."""
                nc.vector.tensor_scalar(
                    out=mu[:, sl], in0=ssum[:, sl], scalar1=1.0 / C,
                    scalar2=None, op0=ALU.mult)
                nc.vector.tensor_tensor(out=nvar[:, sl], in0=mu[:, sl],
                                        in1=mu[:, sl], op=ALU.mult)
                nc.vector.scalar_tensor_tensor(
                    out=nvar[:, sl], in0=sqs[:, sl], scalar=1.0 / C,
                    in1=nvar[:, sl], op0=ALU.mult, op1=ALU.subtract)
                nc.vector.tensor_scalar(
                    out=nvar[:, sl], in0=nvar[:, sl], scalar1=EPS,
                    scalar2=None, op0=ALU.add)
                nc.vector.reciprocal(nvar[:, sl], nvar[:, sl])
                nc.scalar.sqrt(rstd[:, sl], nvar[:, sl])

            def ln1_apply_chunk(b, tci, hT):
                """Normalize chunk and transpose into hT (packed evac)."""
                s = slice(b * NTC + tci, b * NTC + tci + 1)
                xbc = xbc_tiles.pop((b, tci))
                hc = st.tile([P, C], bf16, tag="h", bufs=2,
                             name=f"hc{b}_{tci}")
                nc.vector.tensor_scalar(
                    out=hc[:], in0=xbc[:], scalar1=mu[:, s],
                    scalar2=rstd[:, s], op0=ALU.subtract, op1=ALU.mult)
                tp = ps.tile([P, KT, P], bf16, tag="tp", bufs=1,
                             name=f"tph{b}_{tci}")
                for kt in range(KT):
                    nc.tensor.transpose(tp[:, kt, :],
                                        hc[:, kt * P:(kt + 1) * P],
                                        id_bf[:])
                nc.vector.tensor_copy(
                    out=hT[:, :, tci * P:(tci + 1) * P], in_=tp[:])

            def qkv_k(b, hT, kT, w_sb, th):
                pp = ps.tile([P, TOWN], f32, tag="big", bufs=3,
                             name=f"k{b}_{th}")
                for kt in range(KT):
                    nc.tensor.matmul(
                        pp[:], w_sb[:, kt, :],
                        hT[:, kt, th * TOWN:(th + 1) * TOWN],
                        start=(kt == 0), stop=(kt == KT - 1))
                ts_ = slice(th * TOWN, (th + 1) * TOWN)
                nc.vector.tensor_copy(out=kT[0:D, 0, ts_], in_=pp[0:D, :])
                nc.vector.tensor_copy(out=kT[D:P, 1, ts_], in_=pp[D:P, :])

            def qkv_q(b, hT, qT, w_sb, bq_sb, th):
                pp = ps.tile([P, TOWN], f32, tag="big", bufs=3,
                             name=f"q{b}_{th}")
                for kt in range(KT):
                    nc.tensor.matmul(
                        pp[:], w_sb[:, kt, :],
                        hT[:, kt, th * TOWN:(th + 1) * TOWN],
                        start=(kt == 0), stop=(kt == KT - 1))
                nc.vector.tensor_scalar(
                    out=qT[:, th * TOWN:(th + 1) * TOWN], in0=pp[:],
                    scalar1=bq_sb[:], scalar2=None, op0=ALU.add)

            def qkv_v(b, hT, v_aug, w_sb, tci):
                pp = ps.tile([P, P], f32, tag="tp", bufs=1,
                             name=f"v{b}_{tci}")
                for kt in range(KT):
                    nc.tensor.matmul(
                        pp[:], hT[:, kt, tci * P:(tci + 1) * P],
                        w_sb[:, kt, :],
                        start=(kt == 0), stop=(kt == KT - 1))
                nc.vector.tensor_copy(
                    out=v_aug[:, tci, :, 0:D],
                    in_=pp[:].rearrange("p (h d) -> p h d", d=D))

            def attn_qc(b, qc, kT, qT, v_aug, attn_sb):
                """Scores+exp for ALL key chunks first (keeps the scalar
                engine's exp stream continuous), then AV matmuls with V as
                the stationary operand (one weight load per key chunk, wide
                moving operand) producing feature-major partials [65, 256]
                that are transposed+normalized at evacuation."""
                if True:
                    aps = [ps.tile([P, D + 1], f32, tag="aps", bufs=4,
                                   name=f"aps{b}_{qc}_{i}")
                           for i in range(4)]
                    pend = []

                    def flush_avs():
                        for kc_, sh_, ex_ in pend:
                            for hl in range(2):
                                for ti in range(2):
                                    nc.tensor.matmul(
                                        aps[hl * 2 + ti][:],
                                        ex_[:, hl, ti * P:(ti + 1) * P],
                                        v_aug[:, 2 * kc_ + sh_, hl, :],
                                        start=(kc_ == 0 and sh_ == 0),
                                        stop=(kc_ == qc and sh_ == 1))
                        pend.clear()

                    for kc in range(qc + 1):
                        for sh in range(2):
                            sc = ps.tile([P, 2, CH], f32, tag="big", bufs=3,
                                         name=f"sc{b}_{qc}_{kc}_{sh}")
                            for hl in range(2):
                                nc.tensor.matmul(
                                    sc[:, hl, :],
                                    kT[:, hl,
                                       kc * CH + sh * P:kc * CH + (sh + 1) * P],
                                    qT[:, qc * CH:(qc + 1) * CH],
                                    start=True, stop=True)
                            ex = st.tile([P, 2, CH], bf16, tag="ex", bufs=10,
                                         name=f"ex{b}_{qc}_{kc}_{sh}")
                            nc.scalar.activation(ex[:], sc[:], ACT_F.Exp,
                                                 bias=0.0, scale=SCALE)
                            if kc == qc:
                                nc.vector.tensor_tensor(
                                    out=ex[:], in0=ex[:], in1=mask[:, sh],
                                    op=ALU.mult)
                            pend.append((kc, sh, ex))
                        if len(pend) >= 8:
                            flush_avs()
                    flush_avs()
                    for ti in range(2):
                        for hl in range(2):
                            rd = st.tile([P, 1], f32, tag="rd", bufs=4,
                                         name=f"rd{b}_{qc}_{ti}_{hl}")
                            nc.vector.reciprocal(rd[:],
                                                 aps[hl * 2 + ti][:, D:D + 1])
                            nc.vector.tensor_scalar(
                                out=attn_sb[:, 2 * qc + ti,
                                            hl * D:(hl + 1) * D],
                                in0=aps[hl * 2 + ti][:, 0:D], scalar1=rd[:],
                                scalar2=None, op0=ALU.mult)

            # ---- persistent tiles ----
            hT_0 = sb.tile([P, KT, T], bf16, tag="TA", name="hT_0")
            kT_0 = sb.tile([P, 2, T], bf16, tag="TK0", name="kT_0")
            qT_0 = sb.tile([P, T], bf16, name="qT_0")
            va_0 = sb.tile([P, NTC, 2, D + 1], bf16, name="va_0")
            hT_1 = sb.tile([P, KT, T], bf16, tag="TA", name="hT_1")
            kT_1 = sb.tile([P, 2, T], bf16, name="kT_1")
            qT_1 = sb.tile([P, T], bf16, name="qT_1")
            va_1 = sb.tile([P, NTC, 2, D + 1], bf16, name="va_1")
            at_0 = sb.tile([P, NTC, P], bf16, name="at_0")
            at_1 = sb.tile([P, NTC, P], bf16, name="at_1")
            afT = sb.tile([P, KT, TOWN], f8, name="afT")
            out1 = sb.tile([P, 4, C], f32, name="out1")
            h2T = sb.tile([P, KT, TOWN], bf16, name="h2T")

            # ---- weights (contiguous, host-prearranged) ----
            wq_sb = sb.tile([P, KT, P], bf16)
            wk_sb = sb.tile([P, KT, P], bf16)
            wv_sb = sb.tile([P, KT, P], bf16)
            bq_sb = sb.tile([P, 1], f32)
            for w_sb, ext in ((wk_sb, wk_ext), (wq_sb, wq_ext),
                              (wv_sb, wv_ext), (bq_sb, bq_ext)):
                nc.sync.dma_start(w_sb[:], ext[:])

            # ---- unified schedule: LN1/QKV/attention for both batches ----
            # Early attention query-chunks are emitted as soon as the K/Q/V
            # slices they read exist, keeping tensor/vector/scalar all busy.
            nc.vector.memset(kT_0[D:P, 0, :], 0.0)
            nc.vector.memset(kT_0[0:D, 1, :], 0.0)
            nc.vector.memset(va_0[:, :, :, D:D + 1], 1.0)

            def stats(b, lo, hi):
                for tci in range(lo, hi):
                    ln1_stats_chunk(b, tci)

            def finish(b, lo, hi):
                ln_group_stats(slice(b * NTC + lo, b * NTC + hi))

            def apply(b, lo, hi, hT):
                for tci in range(lo, hi):
                    ln1_apply_chunk(b, tci, hT)

            def vs(b, hT, va, w_sb, lo, hi):
                for tci in range(lo, hi):
                    qkv_v(b, hT, va, w_sb, tci)

            # a2a input buffers (fp8, asymmetric token split: the first
            # collective pays the CC ramp, so keep it small)
            A2A0 = 128
            a2a_in = [dram.tile([NCORES * P, A2A0], f8, name="a2ain0"),
                      dram.tile([NCORES * P, TOWN - A2A0], f8,
                                name="a2ain1")]
            a2a_out = [dram.tile([NCORES * P, A2A0], f8, name="a2aout0"),
                       dram.tile([NCORES * P, TOWN - A2A0], f8,
                                 name="a2aout1")]

            def a2a_send_block(b, attn_sb, ib):
                # transpose one 512-token block of batch-b attention and
                # stage it; global block i = b*4 + ib -> rows i*128..
                i = b * 4 + ib
                atT = st.tile([P, TOWN], f8, tag="atT", bufs=3, name=f"atT{i}")
                tp = ps.tile([P, 4, P], bf16, tag="tp", bufs=1,
                             name=f"tpa{i}")
                for tt in range(4):
                    nc.tensor.transpose(tp[:, tt, :],
                                        attn_sb[:, ib * 4 + tt, :],
                                        id_bf[:])
                nc.vector.tensor_copy(out=atT[:], in_=tp[:])
                nc.sync.dma_start(a2a_in[0][i * P:(i + 1) * P, :],
                                  atT[:, 0:A2A0])
                nc.sync.dma_start(a2a_in[1][i * P:(i + 1) * P, :],
                                  atT[:, A2A0:TOWN])

            def A0(qc):
                attn_qc(0, qc, kT_0, qT_0, va_0, at_0)

            def A1(qc):
                attn_qc(1, qc, kT_1, qT_1, va_1, at_1)

            # schedule (v4b layout): one attention qc per slot, compact
            # build blocks between; batch-1 build rides attention(0)'s tail
            stats(0, 0, 4)
            finish(0, 0, 4)
            apply(0, 0, 4, hT_0)
            stats(0, 4, 8)
            qkv_k(0, hT_0, kT_0, wk_sb, 0)
            qkv_q(0, hT_0, qT_0, wq_sb, bq_sb, 0)
            vs(0, hT_0, va_0, wv_sb, 0, 2)
            A0(0)
            finish(0, 4, 8)
            apply(0, 4, 8, hT_0)
            qkv_k(0, hT_0, kT_0, wk_sb, 1)
            qkv_q(0, hT_0, qT_0, wq_sb, bq_sb, 1)
            vs(0, hT_0, va_0, wv_sb, 2, 4)
            A0(1)
            stats(0, 8, 16)
            finish(0, 8, 16)
            apply(0, 8, 12, hT_0)
            qkv_k(0, hT_0, kT_0, wk_sb, 2)
            qkv_q(0, hT_0, qT_0, wq_sb, bq_sb, 2)
            vs(0, hT_0, va_0, wv_sb, 4, 8)
            A0(2)
            A0(3)
            apply(0, 12, 16, hT_0)
            qkv_k(0, hT_0, kT_0, wk_sb, 3)
            qkv_q(0, hT_0, qT_0, wq_sb, bq_sb, 3)
            vs(0, hT_0, va_0, wv_sb, 8, 16)

            # remaining weights (needed much later)
            wp_sb = sb.tile([P, KT, C], bf16)
            nc.sync.dma_start(wp_sb[:], wp_ext[:])
            xo_sb = sb.tile([P, 4, C], f32)
            nc.sync.dma_start(xo_sb[:], xo_ext[:])
            b1_sb = sb.tile([P, FMT], f32)
            nc.sync.dma_start(b1_sb[:], b1_ext[:])
            b2_sb = sb.tile([P, KT], f32)
            nc.sync.dma_start(b2_sb[:], b2_ext[:])

            A0(4)
            stats(1, 0, 4)
            A0(5)
            stats(1, 4, 8)
            finish(1, 0, 8)
            apply(1, 0, 4, hT_1)
            A0(6)
            stats(1, 8, 12)
            apply(1, 4, 8, hT_1)
            nc.vector.memset(kT_1[D:P, 0, :], 0.0)
            nc.vector.memset(kT_1[0:D, 1, :], 0.0)
            nc.vector.memset(va_1[:, :, :, D:D + 1], 1.0)
            qkv_k(1, hT_1, kT_1, wk_sb, 0)
            qkv_q(1, hT_1, qT_1, wq_sb, bq_sb, 0)
            A0(7)
            stats(1, 12, 16)
            finish(1, 8, 16)
            apply(1, 8, 12, hT_1)
            qkv_k(1, hT_1, kT_1, wk_sb, 1)
            qkv_q(1, hT_1, qT_1, wq_sb, bq_sb, 1)
            vs(1, hT_1, va_1, wv_sb, 0, 4)
            for ib in range(4):
                a2a_send_block(0, at_0, ib)
            apply(1, 12, 16, hT_1)
            A1(0)
            A1(1)
            qkv_k(1, hT_1, kT_1, wk_sb, 2)
            qkv_q(1, hT_1, qT_1, wq_sb, bq_sb, 2)
            vs(1, hT_1, va_1, wv_sb, 4, 10)
            A1(2)
            A1(3)
            qkv_k(1, hT_1, kT_1, wk_sb, 3)
            qkv_q(1, hT_1, qT_1, wq_sb, bq_sb, 3)
            vs(1, hT_1, va_1, wv_sb, 10, 16)
            A1(4)
            A1(5)
            A1(6)
            A1(7)
            for ib in range(4):
                a2a_send_block(1, at_1, ib)
            for h in range(2):
                nc.gpsimd.collective_compute(
                    "AllToAll", ALU.bypass, ins=[a2a_in[h].opt()],
                    outs=[a2a_out[h].opt()], replica_groups=GROUP8)
            for s_i in range(NCORES):
                nc.sync.dma_start(afT[:, s_i, 0:A2A0],
                                  a2a_out[0][s_i * P:(s_i + 1) * P, :])
            for s_i in range(NCORES):
                nc.sync.dma_start(afT[:, s_i, A2A0:TOWN],
                                  a2a_out[1][s_i * P:(s_i + 1) * P, :])

            # ---- proj + residual (own tokens, token-major) ----
            for m in range(4):
                for nh in range(2):
                    cs = slice(nh * TOWN, (nh + 1) * TOWN)
                    pp = ps.tile([P, TOWN], f32, tag="big", bufs=3,
                                 name=f"proj{m}_{nh}")
                    for fc in range(KT):
                        nc.tensor.matmul(
                            pp[:], afT[:, fc, m * P:(m + 1) * P],
                            wp_sb[:, fc, cs],
                            start=(fc == 0), stop=(fc == KT - 1))
                    nc.vector.scalar_tensor_tensor(
                        out=out1[:, m, cs], in0=pp[:], scalar=0.0,
                        in1=xo_sb[:, m, cs], op0=ALU.add, op1=ALU.add)

            # ---- LN2 (own 512 tokens) -> h2T (fp8) ----
            ssum2 = sb.tile([P, 4], f32)
            sqs2 = sb.tile([P, 4], f32)
            mu2 = sb.tile([P, 4], f32)
            rstd2 = sb.tile([P, 4], f32)
            nvar2 = sb.tile([P, 4], f32)
            for m in range(4):
                s = slice(m, m + 1)
                nc.vector.tensor_reduce(ssum2[:, s], out1[:, m, :], AX.X,
                                        ALU.add)
                sqo = st.tile([P, C], bf16, tag="sq", bufs=2,
                              name=f"sqo2_{m}")
                nc.scalar.activation(sqo[:], out1[:, m, :], ACT_F.Square,
                                     accum_out=sqs2[:, s])
            nc.vector.tensor_scalar(
                out=mu2[:], in0=ssum2[:], scalar1=1.0 / C,
                scalar2=None, op0=ALU.mult)
            nc.vector.tensor_tensor(out=nvar2[:], in0=mu2[:],
                                    in1=mu2[:], op=ALU.mult)
            nc.vector.scalar_tensor_tensor(
                out=nvar2[:], in0=sqs2[:], scalar=1.0 / C,
                in1=nvar2[:], op0=ALU.mult, op1=ALU.subtract)
            nc.vector.tensor_scalar(
                out=nvar2[:], in0=nvar2[:], scalar1=EPS,
                scalar2=None, op0=ALU.add)
            nc.vector.reciprocal(nvar2[:], nvar2[:])
            nc.scalar.sqrt(rstd2[:], nvar2[:])
            for m in range(4):
                s = slice(m, m + 1)
                h2c = st.tile([P, C], bf16, tag="h", bufs=2,
                              name=f"h2c{m}")
                nc.vector.tensor_scalar(
                    out=h2c[:], in0=out1[:, m, :], scalar1=mu2[:, s],
                    scalar2=rstd2[:, s], op0=ALU.subtract, op1=ALU.mult)
                for g in range(2):
                    tp = ps.tile([P, 4, P], bf16, tag="tp", bufs=1,
                                 name=f"tph2_{m}_{g}")
                    for k in range(4):
                        kt = g * 4 + k
                        nc.tensor.transpose(tp[:, k, :],
                                            h2c[:, kt * P:(kt + 1) * P],
                                            id_bf[:])
                    nc.vector.tensor_copy(
                        out=h2T[:, g * 4:(g + 1) * 4, m * P:(m + 1) * P],
                        in_=tp[:])

            # ---- FFN1: ff1T = relu(W1.T h2T + b1) ----
            ff1T = sb.tile([P, FMT, TOWN], bf16, tag="TA", name="ff1T")
            for mt in range(FMT):
                w1s = st.tile([P, KT, P], bf16, tag="w1", name=f"w1s{mt}")
                nc.sync.dma_start(w1s[:], w1_ext[mt])
                pp = ps.tile([P, TOWN], f32, tag="big", bufs=3,
                             name=f"ff1{mt}")
                for kt in range(KT):
                    nc.tensor.matmul(pp[:], w1s[:, kt, :], h2T[:, kt, :],
                                     start=(kt == 0), stop=(kt == KT - 1))
                nc.scalar.activation(ff1T[:, mt, :], pp[:], ACT_F.Relu,
                                     bias=b1_sb[:, mt:mt + 1])

            # ---- out1T (+b2), cout-major residual-2 ----
            out1T = sb.tile([P, KT, TOWN], f32, tag="TK0", name="out1T")
            for cc in range(KT):
                tp = ps.tile([P, 4, P], f32, tag="tp", bufs=1,
                             name=f"tpo{cc}")
                for m in range(4):
                    nc.tensor.transpose(tp[:, m, :],
                                        out1[:, m, cc * P:(cc + 1) * P],
                                        id_f32[:])
                nc.vector.tensor_scalar(
                    out=out1T[:, cc, :],
                    in0=tp[:].rearrange("p m t -> p (m t)"),
                    scalar1=b2_sb[:, cc:cc + 1], scalar2=None, op0=ALU.add)

            # ---- FFN2: outT = ff1T.T@W2 + (out1T + b2) ----
            for cc in range(KT):
                w2h = []
                for hh in range(2):
                    w2s = st.tile([P, FMT // 2, P], bf16,
                                  tag=("w2a" if hh == 0 else "w2b"), bufs=1,
                                  name=f"w2s{cc}_{hh}")
                    nc.sync.dma_start(
                        w2s[:], w2_ext[cc, :, hh * (FMT // 2):
                                       (hh + 1) * (FMT // 2), :])
                    w2h.append(w2s)
                pp = ps.tile([P, TOWN], f32, tag="big", bufs=3,
                             name=f"ff2{cc}")
                for kt in range(FMT):
                    nc.tensor.matmul(pp[:], w2h[kt // 16][:, kt % 16, :],
                                     ff1T[:, kt, :],
                                     start=(kt == 0), stop=(kt == FMT - 1))
                ob = st.tile([P, TOWN], f32, tag="ev", bufs=2,
                             name=f"ob{cc}")
                nc.vector.scalar_tensor_tensor(
                    out=ob[:], in0=pp[:], scalar=0.0,
                    in1=out1T[:, cc, :], op0=ALU.add, op1=ALU.add)
                nc.sync.dma_start(outT_ext[cc * P:(cc + 1) * P, :], ob[:])


_NC_CACHE = None


def _get_nc():
    global _NC_CACHE
    if _NC_CACHE is None:
        _NC_CACHE = build()
    return _NC_CACHE


def shard_inputs(x, Wq, Wk, Wv, Wproj, bproj, W1, b1, W2, b2,
                 ln1_w, ln1_b, ln2_w, ln2_b):
    bf = mybir.dt.np(bf16)
    f8n = mybir.dt.np(f8)
    x = np.asarray(x, np.float32)
    # fold LN1 gamma into Wq/Wk/Wv rows; LN2 gamma into W1 rows
    Wqf = (ln1_w[:, None] * Wq).astype(np.float32)
    Wkf = (ln1_w[:, None] * Wk).astype(np.float32)
    Wvf = (ln1_w[:, None] * Wv).astype(np.float32)
    W1f = (ln2_w[:, None] * W1).astype(np.float32)
    bqf = ln1_b @ Wq                       # query bias (kept)
    bvf = ln1_b @ Wv                       # value bias -> folds via Wproj
    b1f = (ln2_b @ W1 + b1).astype(np.float32)
    # residual-1 base addend: bproj + (value-bias term through proj)
    res_add = (bproj + bvf @ Wproj).astype(np.float32)

    xb = np.ascontiguousarray(x).astype(bf)
    # pre-arranged layouts: [kp, kt, cols]
    wp_b = np.ascontiguousarray(
        Wproj.reshape(KT, P, C).transpose(1, 0, 2)).astype(bf)
    w1_8 = np.ascontiguousarray(
        W1f.reshape(KT, P, FMT, P).transpose(2, 1, 0, 3)).astype(bf)
    w2_8 = np.ascontiguousarray(
        np.asarray(W2, np.float32).reshape(FMT, P, KT, P)
        .transpose(2, 1, 0, 3)).astype(bf)
    b1_r = np.ascontiguousarray(b1f.reshape(FMT, P).T, dtype=np.float32)
    b2_r = np.ascontiguousarray(
        np.asarray(b2, np.float32).reshape(KT, P).T, dtype=np.float32)

    in_maps = []
    for c in range(NCORES):
        b, j = c // 4, c % 4
        hs = slice(P * c, P * (c + 1))
        xo = (x[b, TOWN * j:TOWN * (j + 1)] + res_add).astype(np.float32)
        in_maps.append({
            "xb": xb,
            "xo": np.ascontiguousarray(
                xo.reshape(4, P, C).transpose(1, 0, 2)),
            "wq": np.ascontiguousarray(
                Wqf[:, hs].reshape(KT, P, P).transpose(1, 0, 2)).astype(bf),
            "wk": np.ascontiguousarray(
                Wkf[:, hs].reshape(KT, P, P).transpose(1, 0, 2)).astype(bf),
            "wv": np.ascontiguousarray(
                Wvf[:, hs].reshape(KT, P, P).transpose(1, 0, 2)).astype(bf),
            "wp": wp_b,
            "w1": w1_8,
            "w2": w2_8,
            "bq": np.ascontiguousarray(bqf[hs, None], dtype=np.float32),
            "b1": b1_r,
            "b2": b2_r,
        })
    return in_maps


def assemble(results):
    out = np.empty((2, T, C), np.float32)
    for c in range(NCORES):
        b, j = c // 4, c % 4
        out[b, TOWN * j:TOWN * (j + 1)] = results[c]["outT"].T
    return out


def kernel(**inputs):
    nc = _get_nc()
    in_maps = shard_inputs(**{k: np.asarray(v) for k, v in inputs.items()})
    res = run_bass_kernel_spmd(nc, in_maps, list(range(NCORES)))
    return assemble(res.results)
